# revision 3
# baseline (speedup 1.0000x reference)
"""Radius-count kernel: fp8e5m2 DoubleRow + 4x row-tiled PE strips.

Same math as kernel_fp8 (4-piece e5m2 decomposition, K=47 logical rows,
packed [24, 2, *] for DoubleRow).  The PE work is spread over four 32-row
strips of the 128x128 array (K_PHYS=24 <= 32): consecutive matmul chunks
cycle strips 0..3, so each chunk's LDWEIGHTS (the serial cost that killed
plain DoubleRow: ~256 cols, no FWL) overlaps the previous chunk's MATMUL
on a different row group via the PE's reorder window.  lhs/rhs live in
SBUF four times, at base partitions 0/32/64/96.

Epilogue: balanced CA=CD=784 blocks, ScalarE Sign(+accum) / VectorE
is_ge(+accum) as before.
"""

import numpy as np
import ml_dtypes

import os

N = 20000
M = 25000
NCORES = 8
NT = 157
NPAD = NT * 128
MS = M // NCORES
CA = int(os.environ.get("KRN_CA", "784"))
CD = int(os.environ.get("KRN_CD", "784"))
REPEAT = int(os.environ.get("KRN_REPEAT", "1"))
SCR_BUFS = int(os.environ.get("KRN_SCRBUFS", "3"))
NSTRIP = int(os.environ.get("KRN_NSTRIP", "4"))
MPAD = 2 * (CA + CD)
NPIECE = 4
IJMAX = 4

_E5 = ml_dtypes.float8_e5m2
_PROGRAMS = {}
LAST_RESULTS = None


def _split_e5(x, n=NPIECE):
    x = np.asarray(x, np.float32)
    out = []
    r = x
    for _ in range(n):
        p = r.astype(_E5).astype(np.float32)
        out.append(p)
        r = (r - p).astype(np.float32)
    return out


def _row_plan():
    rows = []
    for c in range(3):
        for i in range(NPIECE):
            for j in range(NPIECE):
                if i + j <= IJMAX:
                    rows.append((2 ** (1 - 4 * (i + j)), ("B", c, i), ("A", c, j)))
    for i in range(NPIECE):
        rows.append((3 * 2 ** (-4 * i), ("NB", i), ("ONE",)))
        rows.append((3 * 2 ** (-4 * i), ("ONE",), ("S", i)))
    rows.sort(key=lambda r: r[0])
    assert len(rows) == 47
    return [(l, r) for _, l, r in rows]


_ROWS = _row_plan()
K_PHYS = 24


def _norm2(p):
    pp = (p * p).astype(np.float32)
    return ((pp[:, 0] + pp[:, 1]) + pp[:, 2]).astype(np.float32)


def _build_lhs(pointcloud):
    b = np.asarray(pointcloud, np.float32)
    nv = b.shape[0]
    B = [_split_e5(2.0 * b[:, c]) for c in range(3)]
    NB = _split_e5(-_norm2(b))
    one = np.ones(nv, np.float32)

    lhs = np.zeros((K_PHYS, 2, NPAD), np.float32)
    for r, (lk, _) in enumerate(_ROWS):
        if lk[0] == "B":
            row = B[lk[1]][lk[2]]
        elif lk[0] == "NB":
            row = NB[lk[1]]
        else:
            row = one
        lhs[r // 2, r % 2, :nv] = row
    return lhs.astype(_E5)


def _build_rhs(padding_shard):
    a = np.asarray(padding_shard, np.float32)
    mv = a.shape[0]
    A = [_split_e5(a[:, c]) for c in range(3)]
    S = _split_e5((np.float32(0.25) - _norm2(a)).astype(np.float32))
    one = np.ones(mv, np.float32)

    rhs = np.zeros((K_PHYS, 2, MPAD), np.float32)
    s0_pos = None
    for r, (_, rk) in enumerate(_ROWS):
        if rk[0] == "A":
            row = A[rk[1]][rk[2]]
        elif rk[0] == "S":
            row = S[rk[1]]
            if rk[1] == 0:
                s0_pos = (r // 2, r % 2)
        else:
            row = one
        rhs[r // 2, r % 2, :mv] = row
    rhs[s0_pos[0], s0_pos[1], mv:] = -1.0
    return rhs.astype(_E5)


def _build_operands(pointcloud, padding_shard):
    return _build_lhs(pointcloud), _build_rhs(padding_shard)


def _get_program(repeat=None):
    if repeat is None:
        repeat = REPEAT
    if repeat in _PROGRAMS:
        return _PROGRAMS[repeat]

    import concourse.bacc as bacc
    import concourse.mybir as mybir
    import concourse.tile as tile

    nc = bacc.Bacc("TRN2", target_bir_lowering=False, debug=False,
                   enable_asserts=False, num_devices=NCORES)
    f32 = mybir.dt.float32
    bf16 = mybir.dt.bfloat16
    fp8 = mybir.dt.float8e5
    lhs_d = nc.dram_tensor("lhs_t", [K_PHYS, 2, NPAD], fp8,
                           kind="ExternalInput").ap()
    rhs_d = nc.dram_tensor("rhs", [K_PHYS, 2, MPAD], fp8,
                           kind="ExternalInput").ap()
    act_d = nc.dram_tensor("actsum", [128, 2 * NT], f32,
                           kind="ExternalOutput").ap()
    dve_d = nc.dram_tensor("dvesum", [128, 2 * NT], f32,
                           kind="ExternalOutput").ap()

    DR = mybir.MatmulPerfMode.DoubleRow

    with tile.TileContext(nc) as tc:
        with tc.tile_pool(name="const", bufs=1) as cpool, \
             tc.tile_pool(name="psA", bufs=2, space="PSUM") as psA, \
             tc.tile_pool(name="psB", bufs=2, space="PSUM") as psB, \
             tc.tile_pool(name="scr", bufs=SCR_BUFS) as scr, \
             tc.tile_pool(name="accp", bufs=1) as accp:
            # lhs/rhs replicated on four 32-partition strips
            lhs_sb = cpool.tile([128, 2, NPAD], fp8)
            rhs_sb = cpool.tile([128, 2, MPAD], fp8)
            for s in range(NSTRIP):
                p0 = 32 * s
                nc.sync.dma_start(out=lhs_sb[p0:p0 + K_PHYS], in_=lhs_d)
                nc.sync.dma_start(out=rhs_sb[p0:p0 + K_PHYS], in_=rhs_d)
            bias_sb = cpool.tile([128, 1], f32)
            nc.vector.memset(bias_sb, 1e-30)

            act_sb = accp.tile([128, 2 * NT], f32)
            dve_sb = accp.tile([128, 2 * NT], f32)

            strip_ctr = [0]

            def fill_psum(ps, t, c0, width):
                o = 0
                while o < width:
                    w = min(512, width - o)
                    s = strip_ctr[0] % NSTRIP
                    strip_ctr[0] += 1
                    p0 = 32 * s
                    nc.tensor.matmul(
                        ps[:, o:o + w],
                        lhs_sb[p0:p0 + K_PHYS, :, t * 128:(t + 1) * 128],
                        rhs_sb[p0:p0 + K_PHYS, :, c0 + o:c0 + o + w],
                        start=True, stop=True, perf_mode=DR,
                        tile_position=(p0, 0))
                    o += w

            def body():
                for t in range(NT):
                    for j in range(2):
                        base = j * (CA + CD)
                        col = 2 * t + j
                        pa = psA.tile([128, CA], f32, tag="pa")
                        fill_psum(pa, t, base, CA)
                        sa = scr.tile([128, CA], bf16, tag="sa")
                        nc.scalar.activation(
                            sa, pa, mybir.ActivationFunctionType.Sign,
                            bias=bias_sb, accum_out=act_sb[:, col:col + 1])
                        pb = psB.tile([128, CD], f32, tag="pb")
                        fill_psum(pb, t, base + CA, CD)
                        sv = scr.tile([128, CD], f32, tag="sv")
                        nc.vector.tensor_scalar(
                            sv, pb, 0.0, 0.0,
                            op0=mybir.AluOpType.is_ge, op1=mybir.AluOpType.add,
                            accum_out=dve_sb[:, col:col + 1])

            if repeat > 1:
                with tc.For_i(0, repeat, 1):
                    body()
            else:
                body()

            nc.sync.dma_start(out=act_d, in_=act_sb)
            nc.sync.dma_start(out=dve_d, in_=dve_sb)
    nc.compile()
    _PROGRAMS[repeat] = nc
    return nc


def kernel(pointcloud, pointcloud_padding):
    global LAST_RESULTS
    from concourse.bass_utils import run_bass_kernel_spmd

    pc = np.asarray(pointcloud, np.float32)
    pad = np.asarray(pointcloud_padding, np.float32)

    lhs = _build_lhs(pc)
    in_maps = [{"lhs_t": lhs, "rhs": _build_rhs(pad[i * MS:(i + 1) * MS])}
               for i in range(NCORES)]

    # the axon device flakes (~2% NRT_EXEC_UNIT_UNRECOVERABLE) on otherwise
    # clean programs; retry with a freshly built program before giving up
    res = None
    for attempt in range(3):
        try:
            nc = _get_program()
            res = run_bass_kernel_spmd(nc, in_maps, core_ids=list(range(NCORES)))
            break
        except Exception:
            if attempt == 2:
                raise
            import time as _time
            _PROGRAMS.clear()
            _time.sleep(10.0)
    LAST_RESULTS = res

    total = np.zeros((128, NT), np.float32)
    for i in range(NCORES):
        A = res.results[i]["actsum"]
        D = res.results[i]["dvesum"]
        total += (A[:, 0::2] + A[:, 1::2] + np.float32(2 * CA)) * np.float32(0.5)
        total += D[:, 0::2] + D[:, 1::2]
    counts = total.T.reshape(-1)[:N]
    return np.rint(counts).astype(np.int32).reshape(N, 1)
